# revision 1
# baseline (speedup 1.0000x reference)
"""Cross-correlation layer kernel for Trainium2 (Bass/Tile), SPMD over 8 cores.

Problem: out[b, k, t] = sum_c x1[b, c, t] * x2p[b, c, t + 2D - k]
with x2p = zero-pad(x2, D) along time, D = 10, k in [0, 21).

Full shapes: x1, x2: [16, 512, 8192] fp32 -> out: [16, 21, 8192] fp32.

Sharding: pure data parallel over batch. Each of the 8 cores gets 2 batches
and computes its [2, 21, 8192] slice locally; host concatenates.

Per-core algorithm:
  Inputs are cast fp32->bf16 during the DMA load (SWDGE cast path); for each
  time block of 128 (t0) the PE accumulates over 4 channel chunks in fp32 PSUM:
      G[u, jj] = sum_c x1[c, t0+u] * x2p[c, t0+jj],  u in [0,128), jj in [0,148)
  The needed outputs are the 21 band diagonals  out[20-d, t0+u] = G[u, u+d].
  A per-partition skewed read is not expressible on-chip (compute-engine and
  DMA access patterns apply the same free offsets to every partition), so G
  blocks are staged into a wide SBUF tile and dumped per half-slab to a DRAM
  scratch, where the diagonal becomes a plain strided pattern: with row
  stride SW2, element (u, blk, d) sits at (SW2+1)*u + 148*blk + d, so ONE
  long read run per row (garbage between the 21-wide windows) covers all 8
  blocks' diagonals with only 128 descriptors per gather. A DVE copy packs
  the [128, (blk, 21)] strided columns, a PE transpose (identity matmul)
  flips to [(blk, d), u], and one DMA writes 512B-contiguous runs into
  out[b, k, :] (negative k-stride realizes k = 20 - d).

  Measured on the 8 axon trn2 cores: ~226-255 us HW exec (loads-only floor
  ~198 us), max rel err ~3.5e-3 vs the fp32 reference.
"""

import numpy as np

import concourse.bass as bass
import concourse.mybir as mybir
import concourse.tile as tile
from concourse import bacc
from concourse.masks import make_identity

D = 10
K = 2 * D + 1  # 21 displacements

F32 = mybir.dt.float32
F32R = mybir.dt.float32r
BF16 = mybir.dt.bfloat16


def build_nc(B, C, T, slab, group, n_cores=8, mode="bf16", do_mm=True, do_extract=True):
    """Build the per-core Bass program for inputs [B, C, T] -> out [B, K, T].

    mode: "bf16" (SWDGE cast loads, bf16 matmul, N=148)
          "f32r" (HWDGE fp32 loads, fp32r matmul, N padded to 256)
    """
    assert C % 128 == 0 and T % slab == 0 and slab % 128 == 0
    nblk_slab = slab // 128
    assert nblk_slab % group == 0
    NCC = C // 128  # channel chunks
    NS = T // slab  # slabs per batch
    NBLK = T // 128  # blocks per batch
    GW = 148  # G width: 128 + 2D
    SW = nblk_slab * GW  # staged G width per slab
    GF = group * K  # gathered free width per group (<=128 for PE transpose)
    assert GF <= 128
    f32r = mode == "f32r"
    in_dt = F32 if f32r else BF16
    # fp32r needs moving dim >= 256 for full rate; extra columns are junk
    MMW = 256 if f32r else GW
    x2w = slab + (128 if f32r else 2 * D)

    nc = bacc.Bacc("TRN2", target_bir_lowering=False, num_devices=n_cores, num_swdge_queues=2)
    x1 = nc.dram_tensor("x1", [B, C, T], F32, kind="ExternalInput")
    x2 = nc.dram_tensor("x2", [B, C, T], F32, kind="ExternalInput")
    out = nc.dram_tensor("out", [B, K, T], F32, kind="ExternalOutput")
    stg_dt = BF16 if not f32r else F32  # staging/dump/gather dtype
    HB = nblk_slab // 2  # blocks per half-slab dump
    SW2 = HB * GW
    # DRAM scratch: per half-slab, the G tiles concatenated ([128, 8*148])
    gdr = nc.dram_tensor("gscratch", [B, NS, 2, 128, SW2], stg_dt)

    with tile.TileContext(nc) as tc:
        with (
            tc.tile_pool(
                name="x1p", bufs=(4 if slab <= 2048 else 2) * NCC
            ) as x1p,
            tc.tile_pool(
                name="x2p", bufs=(3 if slab <= 2048 else 2) * NCC
            ) as x2p,
            tc.tile_pool(name="gsb", bufs=3) as gsbp,
            tc.tile_pool(name="diag", bufs=3) as diagp,
            tc.tile_pool(name="outp", bufs=4) as outp,
            tc.tile_pool(name="const", bufs=1) as constp,
            tc.tile_pool(name="ps", bufs=6, space="PSUM") as psp,
            tc.tile_pool(name="pst", bufs=2, space="PSUM") as pstp,
        ):
            ident = constp.tile([128, 128], stg_dt)
            make_identity(nc, ident[:, :])

            for b in range(B):
                for s in range(NS):
                    ts0 = s * slab  # slab start time
                    # ---- load input slabs (SWDGE: casts fp32->bf16 inline) --
                    x1t = [
                        x1p.tile([128, slab], in_dt, name="x1s", tag="x1s")
                        for _ in range(NCC)
                    ]
                    x2t = [
                        x2p.tile([128, x2w], in_dt, name="x2s", tag="x2s")
                        for _ in range(NCC)
                    ]
                    ldeng = nc.sync if f32r else nc.gpsimd
                    for cc in range(NCC):
                        c0 = cc * 128
                        ldeng.dma_start(
                            x1t[cc][:, :], x1[b, c0 : c0 + 128, ts0 : ts0 + slab]
                        )
                        # x2 tile covers x2 time range [ts0 - D, ts0 - D + x2w)
                        lo = ts0 - D
                        lo_c = max(0, lo)
                        hi_c = min(T, lo + x2w)
                        if lo_c > lo:
                            nc.vector.memset(x2t[cc][:, 0 : lo_c - lo], 0.0)
                        if hi_c < lo + x2w:
                            nc.vector.memset(x2t[cc][:, hi_c - lo :], 0.0)
                        ldeng.dma_start(
                            x2t[cc][:, lo_c - lo : hi_c - lo],
                            x2[b, c0 : c0 + 128, lo_c:hi_c],
                        )

                    # ---- per 128-block: matmuls -> G psum -> staging tile ----
                    gsb = gsbp.tile([128, SW], stg_dt, name="gsb", tag="gsb")
                    for blk in range(nblk_slab if do_mm else 0):
                        u0 = blk * 128
                        gps = psp.tile([128, MMW], F32, tag="gps")
                        for cc in range(NCC):
                            lhs = x1t[cc][:, u0 : u0 + 128]
                            rhs = x2t[cc][:, u0 : u0 + MMW]
                            if f32r:
                                lhs = lhs.bitcast(F32R)
                                rhs = rhs.bitcast(F32R)
                            nc.tensor.matmul(
                                gps[:, :],
                                lhs,
                                rhs,
                                start=(cc == 0),
                                stop=(cc == NCC - 1),
                            )
                        nc.vector.tensor_copy(
                            gsb[:, blk * GW : (blk + 1) * GW], gps[:, 0:GW]
                        )
                    # half-slab dumps + gathers: one long run per u covering
                    # 8 blocks' diagonal windows (garbage between windows)
                    dviews = []
                    for h in range(2 if do_extract else 0):
                        nc.sync.dma_start(
                            gdr[b, s, h], gsb[:, h * SW2 : (h + 1) * SW2]
                        )
                        RW = GW * (HB - 1) + K  # run width per u
                        dtile = diagp.tile(
                            [128, SW2], stg_dt, name="dt", tag="diag"
                        )
                        src = bass.AP(
                            gdr,
                            ((b * NS + s) * 2 + h) * 128 * SW2,
                            [[SW2 + 1, 128], [1, RW]],
                        )
                        nc.scalar.dma_start(dtile[:, 0:RW], src)
                        # dtile[u, GW*bb + d] = G_bb[u, u+d]
                        dviews.append(dtile.rearrange("p (bb j) -> p bb j", j=GW))
                    # ---- per group: pack strided cols, transpose, store ----
                    for g in range(nblk_slab // group if do_extract else 0):
                        gpH = HB // group  # groups per half
                        dview = dviews[g // gpH]
                        gl = g % gpH
                        # pack [128, (group, K)] strided cols -> contiguous
                        pk = outp.tile([128, GF], stg_dt, name="pk", tag="pk")
                        nc.vector.tensor_copy(
                            pk[:, :], dview[:, gl * group : (gl + 1) * group, 0:K]
                        )
                        tps = pstp.tile([GF, 128], stg_dt, tag="tps")
                        nc.tensor.transpose(tps[:, :], pk[:, :], ident[:, :])
                        osb = outp.tile([GF, 128], F32, tag="osb")
                        nc.vector.tensor_copy(osb[:, :], tps[:, :])
                        # out[b, 20-d, t0 + blkd*128 + u] ; iterate (blkd, d, u)
                        blk0 = s * nblk_slab + g * group
                        dst = bass.AP(
                            out,
                            (b * K + 2 * D) * T + blk0 * 128,
                            [[128, group], [-T, K], [1, 128]],
                        )
                        nc.sync.dma_start(dst, osb[:, :])

            if not do_extract:
                dummy = constp.tile([128, 16], F32, name="dummy")
                nc.vector.memset(dummy[:, :], 0.0)
                nc.sync.dma_start(
                    bass.AP(out, 0, [[16, 128], [1, 16]]), dummy[:, :]
                )

    nc.compile()
    return nc


_NC_CACHE = {}


def _get_nc(B, C, T, slab, group, n_cores, mode):
    key = (B, C, T, slab, group, n_cores, mode)
    if key not in _NC_CACHE:
        _NC_CACHE[key] = build_nc(B, C, T, slab, group, n_cores=n_cores, mode=mode)
    return _NC_CACHE[key]


def run_sharded(x1, x2, slab=4096, group=4, mode="bf16", trace=False, **spmd_kwargs):
    """Run the SPMD kernel on 8 cores over full inputs; returns (out, results)."""
    from concourse.bass_utils import run_bass_kernel_spmd

    n_cores = 8
    Bf, C, T = x1.shape
    assert Bf % n_cores == 0
    Bs = Bf // n_cores
    nc = _get_nc(Bs, C, T, slab, group, n_cores, mode)
    in_maps = [
        {
            "x1": np.ascontiguousarray(x1[i * Bs : (i + 1) * Bs]),
            "x2": np.ascontiguousarray(x2[i * Bs : (i + 1) * Bs]),
        }
        for i in range(n_cores)
    ]
    res = run_bass_kernel_spmd(
        nc, in_maps, core_ids=list(range(n_cores)), trace=trace, **spmd_kwargs
    )
    out = np.concatenate([r["out"] for r in res.results], axis=0)
    return out, res


def kernel(x1, x2):
    x1 = np.asarray(x1, dtype=np.float32)
    x2 = np.asarray(x2, dtype=np.float32)
    out, _ = run_sharded(x1, x2)
    return out



# revision 12
# speedup vs baseline: 1.1544x; 1.1544x over previous
"""Cross-correlation layer kernel for Trainium2 (Bass/Tile), SPMD over 8 cores.

Problem: out[b, k, t] = sum_c x1[b, c, t] * x2p[b, c, t + 2D - k]
with x2p = zero-pad(x2, D) along time, D = 10, k in [0, 21).

Full shapes: x1, x2: [16, 512, 8192] fp32 -> out: [16, 21, 8192] fp32.

Sharding: pure data parallel over batch. Each of the 8 cores gets 2 batches
and computes its [2, 21, 8192] slice locally; host concatenates.

Per-core algorithm:
  Inputs are cast fp32->bf16 during the DMA load (SWDGE cast path); for each
  time block of 128 (t0) the PE accumulates over 4 channel chunks in fp32 PSUM:
      G[u, jj] = sum_c x1[c, t0+u] * x2p[c, t0+jj],  u in [0,128), jj in [0,148)
  The needed outputs are the 21 band diagonals  out[20-d, t0+u] = G[u, u+d].
  A per-partition skewed read is not expressible on-chip (compute-engine and
  DMA access patterns apply the same free offsets to every partition), so G
  blocks are staged into a wide SBUF tile and dumped per half-slab to a DRAM
  scratch, where the diagonal becomes a plain strided pattern: with row
  stride SW2, element (u, blk, d) sits at (SW2+1)*u + 148*blk + d, so ONE
  long read run per row (garbage between the 21-wide windows) covers all 8
  blocks' diagonals with only 128 descriptors per gather. A DVE copy packs
  the [128, (blk, 21)] strided columns, a PE transpose (identity matmul)
  flips to [(blk, d), u], and one DMA writes 512B-contiguous runs into
  out[b, k, :] (negative k-stride realizes k = 20 - d).

  Measured on the 8 axon trn2 cores: ~226-255 us HW exec (loads-only floor
  ~198 us), max rel err ~3.5e-3 vs the fp32 reference.
"""

import numpy as np

import concourse.bass as bass
import concourse.mybir as mybir
import concourse.tile as tile
from concourse import bacc
from concourse.masks import make_identity

D = 10
K = 2 * D + 1  # 21 displacements

F32 = mybir.dt.float32
F32R = mybir.dt.float32r
BF16 = mybir.dt.bfloat16


def build_nc(B, C, T, slab, group, n_cores=8, mode="bf16", do_mm=True, do_extract=True):
    """Build the per-core Bass program for inputs [B, C, T] -> out [B, K, T].

    mode: "bf16" (SWDGE cast loads, bf16 matmul, N=148)
          "f32r" (HWDGE fp32 loads, fp32r matmul, N padded to 256)
    """
    assert C % 128 == 0 and T % slab == 0 and slab % 128 == 0
    nblk_slab = slab // 128
    assert nblk_slab % group == 0
    NCC = C // 128  # channel chunks
    NS = T // slab  # slabs per batch
    NBLK = T // 128  # blocks per batch
    GW = 148  # G width: 128 + 2D
    SW = nblk_slab * GW  # staged G width per slab
    GF = group * K  # gathered free width per group (<=128 for PE transpose)
    assert GF <= 128
    f32r = mode == "f32r"
    in_dt = F32 if f32r else BF16
    # fp32r needs moving dim >= 256 for full rate; extra columns are junk
    MMW = 256 if f32r else GW
    x2w = slab + (128 if f32r else 2 * D)

    nc = bacc.Bacc("TRN2", target_bir_lowering=False, num_devices=n_cores, num_swdge_queues=2)
    x1 = nc.dram_tensor("x1", [B, C, T], F32, kind="ExternalInput")
    x2 = nc.dram_tensor("x2", [B, C, T], F32, kind="ExternalInput")
    out = nc.dram_tensor("out", [B, K, T], F32, kind="ExternalOutput")
    stg_dt = BF16 if not f32r else F32  # staging/dump/gather dtype
    HB = nblk_slab // 2  # blocks per half-slab dump
    SW2 = HB * GW
    # DRAM scratch: per half-slab, the G tiles concatenated ([128, 8*148])
    gdr = nc.dram_tensor("gscratch", [B, NS, 2, 128, SW2], stg_dt)

    with tile.TileContext(nc) as tc:
        with (
            tc.tile_pool(
                name="x1p", bufs=(4 if slab <= 2048 else 2) * NCC
            ) as x1p,
            tc.tile_pool(
                name="x2p", bufs=(3 if slab <= 2048 else 2) * NCC
            ) as x2p,
            tc.tile_pool(name="gsb", bufs=3) as gsbp,
            tc.tile_pool(name="diag", bufs=3) as diagp,
            tc.tile_pool(name="outp", bufs=4) as outp,
            tc.tile_pool(name="const", bufs=1) as constp,
            tc.tile_pool(name="ps", bufs=6, space="PSUM") as psp,
            tc.tile_pool(name="pst", bufs=2, space="PSUM") as pstp,
        ):
            ident = constp.tile([128, 128], stg_dt)
            make_identity(nc, ident[:, :])

            for b in range(B):
                for s in range(NS):
                    ts0 = s * slab  # slab start time
                    # ---- load input slabs (SWDGE: casts fp32->bf16 inline) --
                    x1t = [
                        x1p.tile([128, slab], in_dt, name="x1s", tag="x1s")
                        for _ in range(NCC)
                    ]
                    x2t = [
                        x2p.tile([128, x2w], in_dt, name="x2s", tag="x2s")
                        for _ in range(NCC)
                    ]
                    ldeng = nc.sync if f32r else nc.gpsimd
                    for cc in range(NCC):
                        c0 = cc * 128
                        ldeng.dma_start(
                            x1t[cc][:, :], x1[b, c0 : c0 + 128, ts0 : ts0 + slab]
                        )
                        # x2 tile covers x2 time range [ts0 - D, ts0 - D + x2w)
                        lo = ts0 - D
                        lo_c = max(0, lo)
                        hi_c = min(T, lo + x2w)
                        if lo_c > lo:
                            nc.vector.memset(x2t[cc][:, 0 : lo_c - lo], 0.0)
                        if hi_c < lo + x2w:
                            nc.vector.memset(x2t[cc][:, hi_c - lo :], 0.0)
                        ldeng.dma_start(
                            x2t[cc][:, lo_c - lo : hi_c - lo],
                            x2[b, c0 : c0 + 128, lo_c:hi_c],
                        )

                    # ---- per 128-block: matmuls -> G psum -> staging tile ----
                    gsb = gsbp.tile([128, SW], stg_dt, name="gsb", tag="gsb")
                    for blk in range(nblk_slab if do_mm else 0):
                        u0 = blk * 128
                        gps = psp.tile([128, MMW], F32, tag="gps")
                        for cc in range(NCC):
                            lhs = x1t[cc][:, u0 : u0 + 128]
                            rhs = x2t[cc][:, u0 : u0 + MMW]
                            if f32r:
                                lhs = lhs.bitcast(F32R)
                                rhs = rhs.bitcast(F32R)
                            nc.tensor.matmul(
                                gps[:, :],
                                lhs,
                                rhs,
                                start=(cc == 0),
                                stop=(cc == NCC - 1),
                            )
                        nc.vector.tensor_copy(
                            gsb[:, blk * GW : (blk + 1) * GW], gps[:, 0:GW]
                        )
                    # half-slab dumps + gathers: one long run per u covering
                    # 8 blocks' diagonal windows (garbage between windows)
                    dviews = []
                    for h in range(2 if do_extract else 0):
                        nc.sync.dma_start(
                            gdr[b, s, h], gsb[:, h * SW2 : (h + 1) * SW2]
                        )
                        RW = GW * (HB - 1) + K  # run width per u
                        dtile = diagp.tile(
                            [128, SW2], stg_dt, name="dt", tag="diag"
                        )
                        src = bass.AP(
                            gdr,
                            ((b * NS + s) * 2 + h) * 128 * SW2,
                            [[SW2 + 1, 128], [1, RW]],
                        )
                        nc.scalar.dma_start(dtile[:, 0:RW], src)
                        # dtile[u, GW*bb + d] = G_bb[u, u+d]
                        dviews.append(dtile.rearrange("p (bb j) -> p bb j", j=GW))
                    # ---- per group: pack strided cols, transpose, store ----
                    for g in range(nblk_slab // group if do_extract else 0):
                        gpH = HB // group  # groups per half
                        dview = dviews[g // gpH]
                        gl = g % gpH
                        # pack [128, (group, K)] strided cols -> contiguous
                        pk = outp.tile([128, GF], stg_dt, name="pk", tag="pk")
                        nc.vector.tensor_copy(
                            pk[:, :], dview[:, gl * group : (gl + 1) * group, 0:K]
                        )
                        tps = pstp.tile([GF, 128], stg_dt, tag="tps")
                        nc.tensor.transpose(tps[:, :], pk[:, :], ident[:, :])
                        osb = outp.tile([GF, 128], F32, tag="osb")
                        nc.vector.tensor_copy(osb[:, :], tps[:, :])
                        # out[b, 20-d, t0 + blkd*128 + u] ; iterate (blkd, d, u)
                        blk0 = s * nblk_slab + g * group
                        dst = bass.AP(
                            out,
                            (b * K + 2 * D) * T + blk0 * 128,
                            [[128, group], [-T, K], [1, 128]],
                        )
                        nc.sync.dma_start(dst, osb[:, :])

            if not do_extract:
                dummy = constp.tile([128, 16], F32, name="dummy")
                nc.vector.memset(dummy[:, :], 0.0)
                nc.sync.dma_start(
                    bass.AP(out, 0, [[16, 128], [1, 16]]), dummy[:, :]
                )

    nc.compile()
    return nc


def build_nc_v2(B, C, T, slab, group, n_cores=8, mode="bf16", dmajor=True):
    """Software-pipelined variant: extraction for slab g is issued two slabs
    behind its matmuls, so the PE stream (and every other engine stream) never
    stalls on the DRAM scratch round-trip; the tail after the last loads is
    just one slab's extraction chain.  One dump+gather per slab (no halves).
    With dmajor=True the pack is displacement-major so the output store's
    innermost runs are group*128 contiguous elements (2KB) instead of 512B.
    """
    assert C % 128 == 0 and T % slab == 0 and slab % 128 == 0
    nblk_slab = slab // 128
    assert nblk_slab % group == 0
    NCC = C // 128
    NS = T // slab
    GW = 148  # G width: 128 + 2D
    SW = nblk_slab * GW
    GF = group * K
    assert GF <= 128
    in_dt = BF16
    MMW = GW
    x2w = slab + 2 * D
    RW = GW * (nblk_slab - 1) + K  # one long gather run per row
    gpg = nblk_slab // group  # groups per slab

    nc = bacc.Bacc(
        "TRN2", target_bir_lowering=False, num_devices=n_cores, num_swdge_queues=2
    )
    x1 = nc.dram_tensor("x1", [B, C, T], F32, kind="ExternalInput")
    x2 = nc.dram_tensor("x2", [B, C, T], F32, kind="ExternalInput")
    out = nc.dram_tensor("out", [B, K, T], F32, kind="ExternalOutput")
    gdr = nc.dram_tensor("gscratch", [B, NS, 128, SW], BF16)

    SL = [(b, s) for b in range(B) for s in range(NS)]
    NG = len(SL)

    with tile.TileContext(nc) as tc:
        with (
            tc.tile_pool(name="x1p", bufs=6 * NCC) as x1p,
            tc.tile_pool(name="x2p", bufs=6 * NCC) as x2p,
            tc.tile_pool(name="gsb", bufs=3) as gsbp,
            tc.tile_pool(name="diag", bufs=3) as diagp,
            tc.tile_pool(name="outp", bufs=6) as outp,
            tc.tile_pool(name="const", bufs=1) as constp,
            tc.tile_pool(name="ps", bufs=6, space="PSUM") as psp,
            tc.tile_pool(name="pst", bufs=2, space="PSUM") as pstp,
        ):
            ident = constp.tile([128, 128], BF16)
            make_identity(nc, ident[:, :])

            loads = {}
            staged = {}

            def issue_loads(g):
                b, s = SL[g]
                ts0 = s * slab
                x1t = [
                    x1p.tile([128, slab], in_dt, name="x1s", tag="x1s")
                    for _ in range(NCC)
                ]
                x2t = [
                    x2p.tile([128, x2w], in_dt, name="x2s", tag="x2s")
                    for _ in range(NCC)
                ]
                for cc in range(NCC):
                    c0 = cc * 128
                    nc.gpsimd.dma_start(
                        x1t[cc][:, :], x1[b, c0 : c0 + 128, ts0 : ts0 + slab]
                    )
                    lo = ts0 - D
                    lo_c = max(0, lo)
                    hi_c = min(T, lo + x2w)
                    if lo_c > lo:
                        nc.vector.memset(x2t[cc][:, 0 : lo_c - lo], 0.0)
                    if hi_c < lo + x2w:
                        nc.vector.memset(x2t[cc][:, hi_c - lo :], 0.0)
                    nc.gpsimd.dma_start(
                        x2t[cc][:, lo_c - lo : hi_c - lo],
                        x2[b, c0 : c0 + 128, lo_c:hi_c],
                    )
                loads[g] = (x1t, x2t)

            def issue_mm(g):
                x1t, x2t = loads.pop(g)
                gsb = gsbp.tile([128, SW], BF16, name="gsb", tag="gsb")
                for blk in range(nblk_slab):
                    u0 = blk * 128
                    gps = psp.tile([128, MMW], F32, tag="gps")
                    for cc in range(NCC):
                        nc.tensor.matmul(
                            gps[:, :],
                            x1t[cc][:, u0 : u0 + 128],
                            x2t[cc][:, u0 : u0 + MMW],
                            start=(cc == 0),
                            stop=(cc == NCC - 1),
                        )
                    nc.vector.tensor_copy(
                        gsb[:, blk * GW : (blk + 1) * GW], gps[:, 0:GW]
                    )
                staged[g] = gsb

            def issue_extract(g):
                b, s = SL[g]
                gsb = staged.pop(g)
                nc.sync.dma_start(gdr[b, s], gsb[:, :])
                dtile = diagp.tile([128, SW], BF16, name="dt", tag="diag")
                src = bass.AP(gdr, (b * NS + s) * 128 * SW, [[SW + 1, 128], [1, RW]])
                nc.scalar.dma_start(dtile[:, 0:RW], src)
                # dtile[u, GW*bb + d] = G_bb[u, u+d]
                for gl in range(gpg):
                    pk = outp.tile([128, GF], BF16, name="pk", tag="pk")
                    if dmajor:
                        # pk[u, d*group+bb] = G_{gl*group+bb}[u, u+d]
                        dv = dtile.rearrange("p (bb j) -> p j bb", j=GW)
                        nc.vector.tensor_copy(
                            pk[:, :], dv[:, 0:K, gl * group : (gl + 1) * group]
                        )
                    else:
                        dv = dtile.rearrange("p (bb j) -> p bb j", j=GW)
                        nc.vector.tensor_copy(
                            pk[:, :], dv[:, gl * group : (gl + 1) * group, 0:K]
                        )
                    tps = pstp.tile([GF, 128], BF16, tag="tps")
                    nc.tensor.transpose(tps[:, :], pk[:, :], ident[:, :])
                    osb = outp.tile([GF, 128], F32, tag="osb")
                    nc.vector.tensor_copy(osb[:, :], tps[:, :])
                    blk0 = s * nblk_slab + gl * group
                    if dmajor:
                        # iterate (d, bb, u): innermost group*128 els contiguous
                        dst = bass.AP(
                            out,
                            (b * K + 2 * D) * T + blk0 * 128,
                            [[-T, K], [128, group], [1, 128]],
                        )
                    else:
                        dst = bass.AP(
                            out,
                            (b * K + 2 * D) * T + blk0 * 128,
                            [[128, group], [-T, K], [1, 128]],
                        )
                    nc.sync.dma_start(dst, osb[:, :])

            for g in range(NG):
                issue_loads(g)
                if g >= 1:
                    issue_mm(g - 1)
                if g >= 2:
                    issue_extract(g - 2)
            issue_mm(NG - 1)
            issue_extract(NG - 2)
            issue_extract(NG - 1)

    nc.compile()
    return nc


def build_nc_v4(B, C, T, slab, SB, n_cores=8):
    """v3 + sub-blocked matmuls to shrink the scratch dump.

    Each 128-t block is computed as 128/SB sub-matmuls of SB lhs columns whose
    rhs window shifts along: G'_q[v, e'] = x1[t0+SB*q+v] . x2[t0+SB*q-D+e'],
    e' in [0, SB+2D).  The staged band is [128, (SB+2D)*IL] instead of
    [128, 148*IL] — (SB+2D)/K-fold write redundancy instead of 148/21.  The
    gather row address becomes affine in (q, v): addr = q*SB*SW + v*(SW+IL)
    + d*IL + bb, still one 2*K*IL-byte run per row.  Everything downstream
    (transpose groups, d-ascending stores, host k-flip) matches v3.
    """
    assert C % 128 == 0 and T % slab == 0 and slab % 128 == 0
    assert 128 % SB == 0
    NQ = 128 // SB  # sub-blocks per 128-t block
    IL = slab // 128
    NCC = C // 128
    NS = T // slab
    GW = SB + 2 * D  # panel width
    SW = IL * GW
    in_dt = BF16
    x2w = slab + 2 * D
    PKW = K * IL

    nc = bacc.Bacc(
        "TRN2", target_bir_lowering=False, num_devices=n_cores, num_swdge_queues=2
    )
    x1 = nc.dram_tensor("x1", [B, C, T], F32, kind="ExternalInput")
    x2 = nc.dram_tensor("x2", [B, C, T], F32, kind="ExternalInput")
    out = nc.dram_tensor("out", [B, K, T], F32, kind="ExternalOutput")
    gdr = nc.dram_tensor("gscratch", [B, NS, 128, SW], BF16)

    SL = [(b, s) for b in range(B) for s in range(NS)]
    NG = len(SL)

    dpt = max(1, 128 // IL)
    tgroups = []
    d0 = 0
    while d0 < K:
        nd = min(dpt, K - d0)
        tgroups.append((d0, nd))
        d0 += nd

    depth = max(2, (6 * 1024) // slab)
    with tile.TileContext(nc) as tc:
        with (
            tc.tile_pool(name="x1p", bufs=depth * NCC) as x1p,
            tc.tile_pool(name="x2p", bufs=depth * NCC) as x2p,
            tc.tile_pool(name="gsb", bufs=3) as gsbp,
            tc.tile_pool(name="diag", bufs=3) as diagp,
            tc.tile_pool(name="outp", bufs=2 * len(tgroups)) as outp,
            tc.tile_pool(name="const", bufs=1) as constp,
            tc.tile_pool(name="ps", bufs=6, space="PSUM") as psp,
            tc.tile_pool(name="pst", bufs=2, space="PSUM") as pstp,
        ):
            ident = constp.tile([128, 128], BF16)
            make_identity(nc, ident[:, :])

            loads = {}
            staged = {}

            def issue_loads(g):
                b, s = SL[g]
                ts0 = s * slab
                x1t = [
                    x1p.tile([128, slab], in_dt, name="x1s", tag="x1s")
                    for _ in range(NCC)
                ]
                x2t = [
                    x2p.tile([128, x2w], in_dt, name="x2s", tag="x2s")
                    for _ in range(NCC)
                ]
                for cc in range(NCC):
                    c0 = cc * 128
                    nc.gpsimd.dma_start(
                        x1t[cc][:, :], x1[b, c0 : c0 + 128, ts0 : ts0 + slab]
                    )
                    lo = ts0 - D
                    lo_c = max(0, lo)
                    hi_c = min(T, lo + x2w)
                    if lo_c > lo:
                        nc.vector.memset(x2t[cc][:, 0 : lo_c - lo], 0.0)
                    if hi_c < lo + x2w:
                        nc.vector.memset(x2t[cc][:, hi_c - lo :], 0.0)
                    nc.gpsimd.dma_start(
                        x2t[cc][:, lo_c - lo : hi_c - lo],
                        x2[b, c0 : c0 + 128, lo_c:hi_c],
                    )
                loads[g] = (x1t, x2t)

            def issue_mm(g):
                x1t, x2t = loads.pop(g)
                gsb = gsbp.tile([128, SW], BF16, name="gsb", tag="gsb")
                gsbv = gsb.rearrange("p (e i) -> p e i", i=IL)
                for blk in range(IL):
                    u0 = blk * 128
                    gps = psp.tile([128, GW], F32, tag="gps")
                    for q in range(NQ):
                        w0 = u0 + q * SB
                        for cc in range(NCC):
                            nc.tensor.matmul(
                                gps[q * SB : (q + 1) * SB, :],
                                x1t[cc][:, w0 : w0 + SB],
                                x2t[cc][:, w0 : w0 + GW],
                                start=(cc == 0),
                                stop=(cc == NCC - 1),
                            )
                    # strided drain: gsb[u, e*IL + blk] = G'[u, e]
                    nc.vector.tensor_copy(
                        gsbv[:, :, blk : blk + 1], gps[:, 0:GW]
                    )
                staged[g] = gsb

            def issue_extract(g):
                b, s = SL[g]
                gsb = staged.pop(g)
                nc.sync.dma_start(gdr[b, s], gsb[:, :])
                dtile = diagp.tile([128, PKW], BF16, name="dt", tag="diag")
                # dtile[u=(q,v), d*IL+bb] = G'_bb_q[v, v+d]
                src = bass.AP(
                    gdr,
                    (b * NS + s) * 128 * SW,
                    [[SB * SW, NQ], [SW + IL, SB], [1, PKW]],
                )
                nc.scalar.dma_start(dtile[:, :], src)
                for d0, nd in tgroups:
                    TW = nd * IL
                    tps = pstp.tile([TW, 128], BF16, tag="tps")
                    nc.tensor.transpose(
                        tps[:, :], dtile[:, d0 * IL : d0 * IL + TW], ident[:, :]
                    )
                    osb = outp.tile([TW, 128], F32, tag="osb")
                    nc.vector.tensor_copy(osb[:, :], tps[:, :])
                    dst = bass.AP(
                        out,
                        (b * K + d0) * T + s * slab,
                        [[T, nd], [128, IL], [1, 128]],
                    )
                    nc.sync.dma_start(dst, osb[:, :])

            for g in range(NG):
                issue_loads(g)
                if g >= 1:
                    issue_mm(g - 1)
                if g >= 2:
                    issue_extract(g - 2)
            issue_mm(NG - 1)
            issue_extract(NG - 2)
            issue_extract(NG - 1)

    nc.compile()
    return nc


def build_nc_v3(B, C, T, slab, n_cores=8):
    """v2 pipeline + interleaved scratch layout.

    The staging tile is written e-major interleaved: gsb[u, e*IL + bb] =
    G_bb[u, e] (IL = blocks per slab), via a strided DVE write during the
    PSUM drain.  The dump stays one contiguous [128, SW] DMA, but the skewed
    DRAM gather now reads, per row u, ONE run of exactly K*IL useful elements
    (addr = u*(SW+IL) + j, j = d*IL + bb), i.e. 148/21 less gather traffic
    and the result is already displacement-major packed.  PE transposes read
    [128, <=128] slices of the gathered tile directly (no DVE pack), and the
    output stores write IL*128-element (4-8KB) contiguous runs per
    displacement.
    """
    assert C % 128 == 0 and T % slab == 0 and slab % 128 == 0
    IL = slab // 128  # blocks per slab = interleave factor
    NCC = C // 128
    NS = T // slab
    GW = 148
    SW = IL * GW
    in_dt = BF16
    x2w = slab + 2 * D
    PKW = K * IL  # packed gather width per row

    nc = bacc.Bacc(
        "TRN2", target_bir_lowering=False, num_devices=n_cores, num_swdge_queues=2
    )
    x1 = nc.dram_tensor("x1", [B, C, T], F32, kind="ExternalInput")
    x2 = nc.dram_tensor("x2", [B, C, T], F32, kind="ExternalInput")
    out = nc.dram_tensor("out", [B, K, T], F32, kind="ExternalOutput")
    gdr = nc.dram_tensor("gscratch", [B, NS, 128, SW], BF16)

    SL = [(b, s) for b in range(B) for s in range(NS)]
    NG = len(SL)

    # transpose column groups: partitions = j = d*IL + bb, <=128 per transpose
    dpt = max(1, 128 // IL)  # displacements per transpose
    tgroups = []
    d0 = 0
    while d0 < K:
        nd = min(dpt, K - d0)
        tgroups.append((d0, nd))
        d0 += nd

    depth = max(2, (6 * 1024) // slab)  # pipeline depth in slabs
    with tile.TileContext(nc) as tc:
        with (
            tc.tile_pool(name="x1p", bufs=depth * NCC) as x1p,
            tc.tile_pool(name="x2p", bufs=depth * NCC) as x2p,
            tc.tile_pool(name="gsb", bufs=3) as gsbp,
            tc.tile_pool(name="diag", bufs=3) as diagp,
            tc.tile_pool(name="outp", bufs=2 * len(tgroups)) as outp,
            tc.tile_pool(name="const", bufs=1) as constp,
            tc.tile_pool(name="ps", bufs=6, space="PSUM") as psp,
            tc.tile_pool(name="pst", bufs=2, space="PSUM") as pstp,
        ):
            ident = constp.tile([128, 128], BF16)
            make_identity(nc, ident[:, :])

            loads = {}
            staged = {}

            def issue_loads(g):
                b, s = SL[g]
                ts0 = s * slab
                x1t = [
                    x1p.tile([128, slab], in_dt, name="x1s", tag="x1s")
                    for _ in range(NCC)
                ]
                x2t = [
                    x2p.tile([128, x2w], in_dt, name="x2s", tag="x2s")
                    for _ in range(NCC)
                ]
                for cc in range(NCC):
                    c0 = cc * 128
                    nc.gpsimd.dma_start(
                        x1t[cc][:, :], x1[b, c0 : c0 + 128, ts0 : ts0 + slab]
                    )
                    lo = ts0 - D
                    lo_c = max(0, lo)
                    hi_c = min(T, lo + x2w)
                    if lo_c > lo:
                        nc.vector.memset(x2t[cc][:, 0 : lo_c - lo], 0.0)
                    if hi_c < lo + x2w:
                        nc.vector.memset(x2t[cc][:, hi_c - lo :], 0.0)
                    nc.gpsimd.dma_start(
                        x2t[cc][:, lo_c - lo : hi_c - lo],
                        x2[b, c0 : c0 + 128, lo_c:hi_c],
                    )
                loads[g] = (x1t, x2t)

            def issue_mm(g):
                x1t, x2t = loads.pop(g)
                gsb = gsbp.tile([128, SW], BF16, name="gsb", tag="gsb")
                gsbv = gsb.rearrange("p (e i) -> p e i", i=IL)
                for blk in range(IL):
                    u0 = blk * 128
                    gps = psp.tile([128, GW], F32, tag="gps")
                    for cc in range(NCC):
                        nc.tensor.matmul(
                            gps[:, :],
                            x1t[cc][:, u0 : u0 + 128],
                            x2t[cc][:, u0 : u0 + GW],
                            start=(cc == 0),
                            stop=(cc == NCC - 1),
                        )
                    # strided drain: gsb[u, e*IL + blk] = G_blk[u, e]
                    nc.vector.tensor_copy(
                        gsbv[:, :, blk : blk + 1], gps[:, 0:GW]
                    )
                staged[g] = gsb

            def issue_extract(g):
                b, s = SL[g]
                gsb = staged.pop(g)
                nc.sync.dma_start(gdr[b, s], gsb[:, :])
                dtile = diagp.tile([128, PKW], BF16, name="dt", tag="diag")
                # dtile[u, d*IL + bb] = G_bb[u, u+d]
                src = bass.AP(gdr, (b * NS + s) * 128 * SW, [[SW + IL, 128], [1, PKW]])
                nc.scalar.dma_start(dtile[:, :], src)
                for d0, nd in tgroups:
                    TW = nd * IL
                    tps = pstp.tile([TW, 128], BF16, tag="tps")
                    nc.tensor.transpose(
                        tps[:, :], dtile[:, d0 * IL : d0 * IL + TW], ident[:, :]
                    )
                    osb = outp.tile([TW, 128], F32, tag="osb")
                    nc.vector.tensor_copy(osb[:, :], tps[:, :])
                    # store d-ASCENDING (out row d holds displacement d; the
                    # host flips k=20-d during unshard).  All strides positive:
                    # negative partition-dim steps are rejected by the BIR
                    # verifier.  Partitions = (d-d0, bb); innermost run is
                    # IL*128 contiguous elements per displacement.
                    dst = bass.AP(
                        out,
                        (b * K + d0) * T + s * slab,
                        [[T, nd], [128, IL], [1, 128]],
                    )
                    nc.sync.dma_start(dst, osb[:, :])

            for g in range(NG):
                issue_loads(g)
                if g >= 1:
                    issue_mm(g - 1)
                if g >= 2:
                    issue_extract(g - 2)
            issue_mm(NG - 1)
            issue_extract(NG - 2)
            issue_extract(NG - 1)

    nc.compile()
    return nc


_NC_CACHE = {}


def _get_nc(B, C, T, slab, group, n_cores, mode, version=2, dmajor=False, sb=128):
    key = (B, C, T, slab, group, n_cores, mode, version, dmajor, sb)
    if key not in _NC_CACHE:
        if version == 4:
            _NC_CACHE[key] = build_nc_v4(B, C, T, slab, sb, n_cores=n_cores)
        elif version == 3:
            _NC_CACHE[key] = build_nc_v3(B, C, T, slab, n_cores=n_cores)
        elif version == 2:
            _NC_CACHE[key] = build_nc_v2(
                B, C, T, slab, group, n_cores=n_cores, mode=mode, dmajor=dmajor
            )
        else:
            _NC_CACHE[key] = build_nc(
                B, C, T, slab, group, n_cores=n_cores, mode=mode
            )
    return _NC_CACHE[key]


def run_sharded(
    x1, x2, slab=1024, group=4, mode="bf16", version=3, dmajor=False, sb=128,
    trace=False, **spmd_kwargs,
):
    """Run the SPMD kernel on 8 cores over full inputs; returns (out, results)."""
    from concourse.bass_utils import run_bass_kernel_spmd

    n_cores = 8
    Bf, C, T = x1.shape
    assert Bf % n_cores == 0
    Bs = Bf // n_cores
    nc = _get_nc(Bs, C, T, slab, group, n_cores, mode, version=version, dmajor=dmajor, sb=sb)
    in_maps = [
        {
            "x1": np.ascontiguousarray(x1[i * Bs : (i + 1) * Bs]),
            "x2": np.ascontiguousarray(x2[i * Bs : (i + 1) * Bs]),
        }
        for i in range(n_cores)
    ]
    res = run_bass_kernel_spmd(
        nc, in_maps, core_ids=list(range(n_cores)), trace=trace, **spmd_kwargs
    )
    out = np.concatenate([r["out"] for r in res.results], axis=0)
    if version >= 3:
        # device stores displacement-major (row d = displacement d); flip to
        # the reference's k = 20 - d ordering during unshard.
        out = np.ascontiguousarray(out[:, ::-1, :])
    return out, res


def kernel(x1, x2):
    x1 = np.asarray(x1, dtype=np.float32)
    x2 = np.asarray(x2, dtype=np.float32)
    out, _ = run_sharded(x1, x2)
    return out

